# revision 5
# baseline (speedup 1.0000x reference)
"""ConvDualAttention Trainium2 kernel (Bass/Tile), 8-core data-parallel.

Contract: kernel(**inputs) takes the FULL unsharded inputs, shards batch b
across the 8 NeuronCores (one batch per core), and returns the full
(8, 128, 4096) float32 output.

Math (per batch b, per head h, D=128, X=4096):
  y_p   = dwconv3(x) + t_p/s_p           (p in q,k,v; BN folded so that
                                          W_eff_p @ y_p == pw_p @ BN(conv))
  k     = W_eff_k @ y_k ; sk = softmax(k over d)
  kat   = SCALE * q^T @ sk               (SCALE folded into W_q)
  gout  = GW @ q + gb ; sig = sigmoid(gout)
  out_h = v @ kat + sig^T * v
  out   = out_w @ merge(out_h) + out_b

v2 kernel structure (all matmuls bf16):
  * yqt (y_q transposed, bias included) comes from a DMA transpose of y_q,
    so kat_h = wtq_h^T @ R_h with R_h = yqt^T @ sk_h -- no sigma/ones path.
  * v@kat through the output projection collapses to W3 @ y_v with
    W3 = sum_h outw_h @ (Wv_h^T @ kat_h)^T, computed on-chip.
  * exp evacuations are 2048 wide (4 PSUM banks); Z row-sums in bf16 on DVE;
    softmax normalize split between DVE and GpSimd; gate production for the
    first chunks is interleaved with K/softmax to keep the PE busy.
"""
import numpy as np
import ml_dtypes

import concourse.bass as bass
import concourse.tile as tile
from concourse import bacc, mybir
from concourse.bass_utils import run_bass_kernel_spmd

F32 = mybir.dt.float32
BF16 = mybir.dt.bfloat16
AF = mybir.ActivationFunctionType
ALU = mybir.AluOpType

B = 8
DIM = 128
HEADS = 8
INNER = DIM * HEADS
X = 4096
EPS = 1e-5
SCALE = DIM ** -0.5
NT = X // 128          # 32 x-tiles of 128
NCH = X // 512         # 8 chunks of 512
GROUPS = 2
GH = HEADS // GROUPS   # 4 heads per group
NQ = NT // 4           # 8 tile-quads per group

_NC = None
TRACE = False
LAST_EXEC_NS = None


def _bf(a):
    return np.ascontiguousarray(np.asarray(a, np.float32).astype(ml_dtypes.bfloat16))


def _prep(inputs):
    """Host-side weight folding. Returns dict of DRAM input arrays."""
    f = lambda k: np.asarray(inputs[k], np.float32)
    wt = {}
    tprime = {}
    diag_cols = []
    for p in ("q", "k", "v"):
        s = f(p + "_g") / np.sqrt(f(p + "_v") + EPS)        # (128,)
        t = f(p + "_b") - f(p + "_m") * s
        tprime[p] = t / s
        w_eff = f(p + "_pw") * s[None, :]                    # (1024, 128)
        wt[p] = np.ascontiguousarray(w_eff.T)                # (128, 1024)
        dw = f(p + "_dw")[:, 0, :]                           # (128, 3)
        for j in range(3):
            diag_cols.append(np.diag(dw[:, j]).astype(np.float32))
    s_gt = f("gt_g") / np.sqrt(f("gt_v") + EPS)
    t_gt = f("gt_b") - f("gt_m") * s_gt
    gw = f("gt_pw") * (f("gt_dw")[:, 0, 0] * s_gt)[None, :]  # (128, 128)
    gb = f("gt_pw") @ t_gt                                   # (128,)
    w_eff_q = wt["q"].T                                      # (1024, 128)
    gqt = np.concatenate(
        [(gw @ w_eff_q[h * 128:(h + 1) * 128, :]).T for h in range(HEADS)], axis=1
    )                                                        # (128 i, 1024 h*o)
    out_w = f("out_w")                                       # (128, 1024)
    outwt = np.concatenate(
        [np.ascontiguousarray(out_w[:, h * 128:(h + 1) * 128].T) for h in range(HEADS)],
        axis=1,
    )                                                        # (128 d, 1024 h*o)
    wvdm = np.concatenate(
        [wt["v"].T[h * 128:(h + 1) * 128, :] for h in range(HEADS)], axis=1
    )                                                        # (128 d, 1024 h*i)
    diag = np.concatenate(diag_cols, axis=1)                 # (128, 1152)
    wtq_s = wt["q"] * SCALE                                  # (128 i, 1024 d)
    biasp = np.stack(
        [tprime["q"], tprime["k"], tprime["v"], gb, f("out_b")], axis=1
    )                                                        # (128, 5)
    return {
        "wtk": _bf(wt["k"]),
        "wtv": _bf(wt["v"]),
        "gqt": _bf(gqt),
        "outwt": _bf(outwt),
        "wvdm": _bf(wvdm),
        "diag": _bf(diag),
        "biasp": np.ascontiguousarray(biasp.astype(np.float32)),
        "wtqr": _bf(wtq_s),
    }


def _build():
    nc = bacc.Bacc("TRN2", target_bir_lowering=False, debug=False, num_devices=B)
    xb_d = nc.dram_tensor("xb", [128, X + 2], BF16, kind="ExternalInput").ap()
    wtk_d = nc.dram_tensor("wtk", [128, INNER], BF16, kind="ExternalInput").ap()
    wtv_d = nc.dram_tensor("wtv", [128, INNER], BF16, kind="ExternalInput").ap()
    gqt_d = nc.dram_tensor("gqt", [128, INNER], BF16, kind="ExternalInput").ap()
    outwt_d = nc.dram_tensor("outwt", [128, INNER], BF16, kind="ExternalInput").ap()
    wvdm_d = nc.dram_tensor("wvdm", [128, INNER], BF16, kind="ExternalInput").ap()
    diag_d = nc.dram_tensor("diag", [128, 9 * 128], BF16, kind="ExternalInput").ap()
    biasp_d = nc.dram_tensor("biasp", [128, 5], F32, kind="ExternalInput").ap()
    wtqr_d = nc.dram_tensor("wtqr", [128, INNER], BF16, kind="ExternalInput").ap()
    out_d = nc.dram_tensor("out", [128, X], F32, kind="ExternalOutput").ap()

    with tile.TileContext(nc) as tc:
        with (
            tc.tile_pool(name="const", bufs=1) as cp,
            tc.tile_pool(name="sks", bufs=2) as skp,
            tc.tile_pool(name="gates", bufs=1) as gp,
            tc.tile_pool(name="gatesjit", bufs=3) as gpj,
            tc.tile_pool(name="sigp", bufs=3) as sgp,
            tc.tile_pool(name="outp", bufs=2) as op_,
        ):
            wtk = cp.tile([128, INNER], BF16)
            wtv = cp.tile([128, INNER], BF16)
            gqt = cp.tile([128, INNER], BF16)
            outwt = cp.tile([128, INNER], BF16)
            wvdm = cp.tile([128, INNER], BF16)
            wtqr = cp.tile([128, INNER], BF16)
            biasp = cp.tile([128, 5], F32)
            xpb = cp.tile([128, X + 2], BF16)
            diag = cp.tile([128, 9 * 128], BF16)
            yq = cp.tile([128, X], BF16, tag="yq")
            yk = cp.tile([128, X], BF16, tag="yk")
            yv = cp.tile([128, X], BF16, tag="yv")
            yqt = cp.tile([128, X], BF16, tag="yqt")
            zt = cp.tile([128, 2 * NT * GH], BF16, tag="zt")
            zi = cp.tile([128, 2 * NT * GH], F32, tag="zi")
            w3t_sb = cp.tile([128, 128], BF16, tag="w3t")

            nc.sync.dma_start(out=xpb, in_=xb_d)
            nc.sync.dma_start(out=diag, in_=diag_d)
            nc.sync.dma_start(out=biasp, in_=biasp_d)
            for sb_t, dr in ((wtk, wtk_d), (wtqr, wtqr_d), (gqt, gqt_d),
                             (wtv, wtv_d), (wvdm, wvdm_d), (outwt, outwt_d)):
                nc.sync.dma_start(out=sb_t, in_=dr)

            ys = {"q": yq, "k": yk, "v": yv}
            yqt3 = yqt.rearrange("p (t j) -> p t j", t=NT)

            # ---- S1: y-stage: depthwise conv via 3 shifted diagonal matmuls
            with tc.tile_pool(name="yps", bufs=4, space="PSUM") as yps:
                for pi, p in enumerate(("q", "k", "v")):
                    for c in range(NCH):
                        pt = yps.tile([128, 512], F32, tag="yps")
                        for j in range(3):
                            dsl = diag[:, (pi * 3 + j) * 128:(pi * 3 + j + 1) * 128]
                            nc.tensor.matmul(
                                pt, dsl,
                                xpb[:, c * 512 + j:c * 512 + j + 512],
                                start=(j == 0), stop=(j == 2),
                            )
                        ysl = ys[p][:, c * 512:(c + 1) * 512]
                        if c % 2 == 0:
                            nc.scalar.activation(
                                ysl, pt, AF.Identity, bias=biasp[:, pi:pi + 1],
                            )
                        else:
                            nc.vector.tensor_scalar(
                                ysl, pt, biasp[:, pi:pi + 1], None, ALU.add,
                            )
                        if p == "q":
                            # yqt tile-major transpose (bias included)
                            nc.sync.dma_start_transpose(
                                yqt3[:, c * 4:(c + 1) * 4, :], ysl,
                            )

            # ---- S2: K + softmax (exp/Z/normalize) interleaved with gate
            #      production for chunks 0..3
            sks = []
            for _g in range(2):
                sksb_g = skp.tile([128, NT * 512], BF16, tag="sksb")
                sks.append(sksb_g)
            gtiles = {}

            def gout_v_unit(c, u, gops, vps):
                """One head-pair unit of phase-B gate production for chunk c."""
                csl = slice(c * 512, (c + 1) * 512)
                g_ps = gops.tile([128, 1024], F32, tag="gout")
                v_ps = vps.tile([128, 1024], F32, tag="vp")
                for d in range(2):
                    h = u * 2 + d
                    nc.tensor.matmul(
                        g_ps[:, d * 512:(d + 1) * 512],
                        gqt[:, h * 128:(h + 1) * 128],
                        yq[:, csl], start=True, stop=True,
                        skip_group_check=True,
                    )
                for d in range(2):
                    h = u * 2 + d
                    nc.tensor.matmul(
                        v_ps[:, d * 512:(d + 1) * 512],
                        wtv[:, h * 128:(h + 1) * 128],
                        yv[:, csl], start=True, stop=True,
                        skip_group_check=True,
                    )
                sig = sgp.tile([128, 1024], BF16, tag="sig")
                nc.scalar.activation(sig, g_ps, AF.Sigmoid, bias=biasp[:, 3:4])
                if c < 4:
                    gate = gp.tile([128, 1024], BF16, tag=f"gate{c}_{u}")
                else:
                    gate = gpj.tile([128, 1024], BF16, tag="gatejit")
                nc.vector.tensor_tensor(gate, v_ps, sig, ALU.mult)
                gtiles[(c, u)] = gate

            def k_quad(g, q, kqps):
                osl = slice(g * 512, (g + 1) * 512)
                sksb = sks[g]
                t0 = q * 4
                kq = kqps.tile([128, 2048], F32, tag="kq")
                for i in range(4):
                    t = t0 + i
                    nc.tensor.matmul(
                        kq[:, i * 512:(i + 1) * 512],
                        yk[:, t * 128:(t + 1) * 128], wtk[:, osl],
                        start=True, stop=True, skip_group_check=True,
                    )
                nc.scalar.activation(
                    sksb[:, t0 * 512:(t0 + 4) * 512], kq, AF.Exp,
                )
                zb = g * NT * GH + t0 * GH
                with nc.allow_low_precision(reason="softmax Z in bf16"):
                    nc.vector.tensor_reduce(
                        zt[:, zb:zb + 4 * GH],
                        sksb[:, t0 * 512:(t0 + 4) * 512].rearrange(
                            "p (t h d) -> p t h d", t=4, h=GH
                        ),
                        mybir.AxisListType.X, ALU.add,
                    )
                nc.vector.reciprocal(zi[:, zb:zb + 4 * GH], zt[:, zb:zb + 4 * GH])
                for i in range(4):
                    t = t0 + i
                    for hh in range(GH):
                        col = g * NT * GH + t * GH + hh
                        sl = sksb[:, t * 512 + hh * 128:t * 512 + (hh + 1) * 128]
                        eng = nc.vector if (i * GH + hh) % 2 == 0 else nc.gpsimd
                        eng.tensor_scalar(sl, sl, zi[:, col:col + 1], None, ALU.mult)

            with (
                tc.tile_pool(name="kqps", bufs=1, space="PSUM") as kqps,
                tc.tile_pool(name="gops1", bufs=1, space="PSUM") as gops1,
                tc.tile_pool(name="vps1", bufs=1, space="PSUM") as vps1,
            ):
                # 16 K-quads round-robined with 16 gate units (chunks 0..3)
                for step in range(16):
                    g, q = divmod(step, NQ)
                    k_quad(g, q, kqps)
                    c, u = divmod(step, 4)
                    gout_v_unit(c, u, gops1, vps1)

            # ---- S3: R accumulation + kat chain -> W3^T
            with (
                tc.tile_pool(name="rps", bufs=2, space="PSUM") as rps,
                tc.tile_pool(name="katps", bufs=1, space="PSUM") as katps,
                tc.tile_pool(name="m2ps", bufs=1, space="PSUM") as m2ps,
                tc.tile_pool(name="w3ps", bufs=1, space="PSUM") as w3ps,
                tc.tile_pool(name="small", bufs=2) as sp,
            ):
                w3t_ps = w3ps.tile([128, 128], F32)
                for g in range(GROUPS):
                    sksb = sks[g]
                    r_ps = rps.tile([128, 512], F32, tag="r")
                    for t in range(NT):
                        nc.tensor.matmul(
                            r_ps, yqt[:, t * 128:(t + 1) * 128],
                            sksb[:, t * 512:(t + 1) * 512],
                            start=(t == 0), stop=(t == NT - 1),
                            skip_group_check=True,
                        )
                    r_sb = sp.tile([128, 512], BF16, tag="rsb")
                    nc.vector.tensor_copy(r_sb, r_ps)
                    for hh in range(GH):
                        h = g * GH + hh
                        kat_ps = katps.tile([128, 128], F32, tag="katp")
                        nc.tensor.matmul(
                            kat_ps, wtqr[:, h * 128:(h + 1) * 128],
                            r_sb[:, hh * 128:(hh + 1) * 128],
                            start=True, stop=True, skip_group_check=True,
                        )
                        kat_sb = sp.tile([128, 128], BF16, tag="katsb")
                        nc.vector.tensor_copy(kat_sb, kat_ps)
                        m2_ps = m2ps.tile([128, 128], F32, tag="m2")
                        nc.tensor.matmul(
                            m2_ps, kat_sb,
                            wvdm[:, h * 128:(h + 1) * 128],
                            start=True, stop=True, skip_group_check=True,
                        )
                        m2_sb = sp.tile([128, 128], BF16, tag="m2sb")
                        nc.vector.tensor_copy(m2_sb, m2_ps)
                        nc.tensor.matmul(
                            w3t_ps, m2_sb,
                            outwt[:, h * 128:(h + 1) * 128],
                            start=(h == 0), stop=(h == HEADS - 1),
                            skip_group_check=True,
                        )
                nc.vector.tensor_copy(w3t_sb, w3t_ps)

            # ---- S4: final projection; JIT gate production for chunks 4..7
            with (
                tc.tile_pool(name="gops2", bufs=2, space="PSUM") as gops2,
                tc.tile_pool(name="vps2", bufs=1, space="PSUM") as vps2,
                tc.tile_pool(name="finps", bufs=2, space="PSUM") as finps,
            ):
                def fin_chunk(c):
                    csl = slice(c * 512, (c + 1) * 512)
                    fin_ps = finps.tile([128, 512], F32, tag="fin")
                    for u in range(4):
                        gate = gtiles[(c, u)]
                        for d in range(2):
                            h = u * 2 + d
                            nc.tensor.matmul(
                                fin_ps, outwt[:, h * 128:(h + 1) * 128],
                                gate[:, d * 512:(d + 1) * 512],
                                start=(h == 0), stop=False,
                                skip_group_check=True,
                            )
                    nc.tensor.matmul(
                        fin_ps, w3t_sb, yv[:, csl],
                        start=False, stop=True, skip_group_check=True,
                    )
                    fin_sb = op_.tile([128, 512], F32, tag="finsb")
                    nc.vector.tensor_scalar(
                        fin_sb, fin_ps, biasp[:, 4:5], None, ALU.add,
                    )
                    nc.sync.dma_start(out=out_d[:, csl], in_=fin_sb)

                # produce chunks 4,5 gates first, then pipeline fins
                for u in range(4):
                    gout_v_unit(4, u, gops2, vps2)
                for u in range(4):
                    gout_v_unit(5, u, gops2, vps2)
                fin_chunk(4)
                for u in range(4):
                    gout_v_unit(6, u, gops2, vps2)
                fin_chunk(5)
                for u in range(4):
                    gout_v_unit(7, u, gops2, vps2)
                for c in (6, 7, 0, 1, 2, 3):
                    fin_chunk(c)

    nc.compile()
    return nc


def kernel(**inputs):
    global _NC, LAST_EXEC_NS
    host = _prep(inputs)
    if _NC is None:
        _NC = _build()
    x = np.asarray(inputs["x"], np.float32)
    in_maps = []
    for b in range(B):
        xp = np.pad(x[b], ((0, 0), (1, 1)))
        m = {"xb": _bf(xp)}
        m.update(host)
        in_maps.append(m)
    res = run_bass_kernel_spmd(
        _NC, in_maps, core_ids=list(range(B)), trace=TRACE
    )
    LAST_EXEC_NS = res.exec_time_ns
    return np.stack([r["out"] for r in res.results]).astype(np.float32)


# revision 10
# speedup vs baseline: 2.0019x; 2.0019x over previous
"""ConvDualAttention Trainium2 kernel (Bass/Tile), 8-core data-parallel.

Contract: kernel(**inputs) takes the FULL unsharded inputs, shards batch b
across the 8 NeuronCores (one batch per core), and returns the full
(8, 128, 4096) float32 output.

Math (per batch b, per head h, D=128, X=4096):
  y_p   = dwconv3(x) + t_p/s_p           (p in q,k,v; BN folded so that
                                          W_eff_p @ y_p == pw_p @ BN(conv))
  k     = W_eff_k @ y_k ; sk = softmax(k over d)
  kat   = SCALE * q^T @ sk               (SCALE folded into W_q)
  gout  = GW @ q + gb ; sig = sigmoid(gout)
  out_h = v @ kat + sig^T * v
  out   = out_w @ merge(out_h) + out_b

v2 kernel structure (all matmuls bf16):
  * yqt (y_q transposed, bias included) comes from a DMA transpose of y_q,
    so kat_h = wtq_h^T @ R_h with R_h = yqt^T @ sk_h -- no sigma/ones path.
  * v@kat through the output projection collapses to W3 @ y_v with
    W3 = sum_h outw_h @ (Wv_h^T @ kat_h)^T, computed on-chip.
  * exp evacuations are 2048 wide (4 PSUM banks); Z row-sums in bf16 on DVE;
    softmax normalize split between DVE and GpSimd; gate production for the
    first chunks is interleaved with K/softmax to keep the PE busy.
"""
import numpy as np
import ml_dtypes

import concourse.bass as bass
import concourse.tile as tile
from concourse import bacc, mybir
from concourse.bass_utils import run_bass_kernel_spmd

F32 = mybir.dt.float32
BF16 = mybir.dt.bfloat16
AF = mybir.ActivationFunctionType
ALU = mybir.AluOpType

B = 8
DIM = 128
HEADS = 8
INNER = DIM * HEADS
X = 4096
EPS = 1e-5
SCALE = DIM ** -0.5
NT = X // 128          # 32 x-tiles of 128
NCH = X // 512         # 8 chunks of 512
GROUPS = 2
GH = HEADS // GROUPS   # 4 heads per group
NQ = NT // 4           # 8 tile-quads per group

_NC = None
TRACE = False
LAST_EXEC_NS = None


def _bf(a):
    return np.ascontiguousarray(np.asarray(a, np.float32).astype(ml_dtypes.bfloat16))


def _prep(inputs):
    """Host-side weight folding. Returns dict of DRAM input arrays."""
    f = lambda k: np.asarray(inputs[k], np.float32)
    wt = {}
    tprime = {}
    diag_cols = []
    for p in ("q", "k", "v"):
        s = f(p + "_g") / np.sqrt(f(p + "_v") + EPS)        # (128,)
        t = f(p + "_b") - f(p + "_m") * s
        tprime[p] = t / s
        w_eff = f(p + "_pw") * s[None, :]                    # (1024, 128)
        wt[p] = np.ascontiguousarray(w_eff.T)                # (128, 1024)
        dw = f(p + "_dw")[:, 0, :]                           # (128, 3)
        for j in range(3):
            diag_cols.append(np.diag(dw[:, j]).astype(np.float32))
    s_gt = f("gt_g") / np.sqrt(f("gt_v") + EPS)
    t_gt = f("gt_b") - f("gt_m") * s_gt
    gw = f("gt_pw") * (f("gt_dw")[:, 0, 0] * s_gt)[None, :]  # (128, 128)
    gb = f("gt_pw") @ t_gt                                   # (128,)
    w_eff_q = wt["q"].T                                      # (1024, 128)
    gqt = np.concatenate(
        [(gw @ w_eff_q[h * 128:(h + 1) * 128, :]).T for h in range(HEADS)], axis=1
    )                                                        # (128 i, 1024 h*o)
    out_w = f("out_w")                                       # (128, 1024)
    outwt = np.concatenate(
        [np.ascontiguousarray(out_w[:, h * 128:(h + 1) * 128].T) for h in range(HEADS)],
        axis=1,
    )                                                        # (128 d, 1024 h*o)
    wvdm = np.concatenate(
        [wt["v"].T[h * 128:(h + 1) * 128, :] for h in range(HEADS)], axis=1
    )                                                        # (128 d, 1024 h*i)
    diag = np.concatenate(diag_cols, axis=1)                 # (128, 1152)
    wtq_s = wt["q"] * SCALE                                  # (128 i, 1024 d)
    # gate path uses sigmoid(x) = 0.5*(1+tanh(x/2)): fold the 0.5 into the
    # phase-B v weights and halve the gate bias (tanh shares the exp table).
    biasp = np.stack(
        [tprime["q"], tprime["k"], tprime["v"], 0.5 * gb, f("out_b")], axis=1
    )                                                        # (128, 5)
    return {
        "wtk": _bf(wt["k"]),
        "wtv": _bf(0.5 * wt["v"]),
        "gqt": _bf(gqt),
        "outwt": _bf(outwt),
        "wvdm": _bf(wvdm),
        "diag": _bf(diag),
        "biasp": np.ascontiguousarray(biasp.astype(np.float32)),
        "wtqr": _bf(wtq_s),
    }


def _build():
    nc = bacc.Bacc("TRN2", target_bir_lowering=False, debug=False, num_devices=B)
    xb_d = nc.dram_tensor("xb", [128, X + 2], BF16, kind="ExternalInput").ap()
    wtk_d = nc.dram_tensor("wtk", [128, INNER], BF16, kind="ExternalInput").ap()
    wtv_d = nc.dram_tensor("wtv", [128, INNER], BF16, kind="ExternalInput").ap()
    gqt_d = nc.dram_tensor("gqt", [128, INNER], BF16, kind="ExternalInput").ap()
    outwt_d = nc.dram_tensor("outwt", [128, INNER], BF16, kind="ExternalInput").ap()
    wvdm_d = nc.dram_tensor("wvdm", [128, INNER], BF16, kind="ExternalInput").ap()
    diag_d = nc.dram_tensor("diag", [128, 9 * 128], BF16, kind="ExternalInput").ap()
    biasp_d = nc.dram_tensor("biasp", [128, 5], F32, kind="ExternalInput").ap()
    wtqr_d = nc.dram_tensor("wtqr", [128, INNER], BF16, kind="ExternalInput").ap()
    out_d = nc.dram_tensor("out", [128, X], F32, kind="ExternalOutput").ap()

    with tile.TileContext(nc) as tc:
        with (
            tc.tile_pool(name="const", bufs=1) as cp,
            tc.tile_pool(name="sks", bufs=2) as skp,
            tc.tile_pool(name="gates", bufs=1) as gp,
            tc.tile_pool(name="gatesjit", bufs=3) as gpj,
            tc.tile_pool(name="sigp", bufs=3) as sgp,
            tc.tile_pool(name="outp", bufs=2) as op_,
        ):
            wtk = cp.tile([128, INNER], BF16)
            wtv = cp.tile([128, INNER], BF16)
            gqt = cp.tile([128, INNER], BF16)
            outwt = cp.tile([128, INNER], BF16)
            wvdm = cp.tile([128, INNER], BF16)
            wtqr = cp.tile([128, INNER], BF16)
            biasp = cp.tile([128, 5], F32)
            xpb = cp.tile([128, X + 2], BF16)
            diag = cp.tile([128, 9 * 128], BF16)
            yq = cp.tile([128, X], BF16, tag="yq")
            yk = cp.tile([128, X], BF16, tag="yk")
            yv = cp.tile([128, X], BF16, tag="yv")
            yqt = cp.tile([128, X], BF16, tag="yqt")
            zt = cp.tile([128, 2 * NT * GH], BF16, tag="zt")
            zi = cp.tile([128, 2 * NT * GH], F32, tag="zi")
            w3t_sb = cp.tile([128, 128], BF16, tag="w3t")

            nc.sync.dma_start(out=xpb, in_=xb_d)
            nc.sync.dma_start(out=diag, in_=diag_d)
            nc.sync.dma_start(out=biasp, in_=biasp_d)
            for sb_t, dr in ((wtk, wtk_d), (wtqr, wtqr_d), (gqt, gqt_d),
                             (wtv, wtv_d), (wvdm, wvdm_d), (outwt, outwt_d)):
                nc.sync.dma_start(out=sb_t, in_=dr)

            ys = {"q": yq, "k": yk, "v": yv}
            yqt3 = yqt.rearrange("p (t j) -> p t j", t=NT)

            # ---- S1: y-stage: depthwise conv via 3 shifted diagonal matmuls
            with tc.tile_pool(name="yps", bufs=4, space="PSUM") as yps:
                for pi, p in enumerate(("q", "k", "v")):
                    for c in range(NCH):
                        pt = yps.tile([128, 512], F32, tag="yps")
                        for j in range(3):
                            dsl = diag[:, (pi * 3 + j) * 128:(pi * 3 + j + 1) * 128]
                            nc.tensor.matmul(
                                pt, dsl,
                                xpb[:, c * 512 + j:c * 512 + j + 512],
                                start=(j == 0), stop=(j == 2),
                            )
                        ysl = ys[p][:, c * 512:(c + 1) * 512]
                        nc.scalar.activation(
                            ysl, pt, AF.Identity, bias=biasp[:, pi:pi + 1],
                        )
                        if p == "q":
                            # yqt tile-major transpose (bias included)
                            nc.sync.dma_start_transpose(
                                yqt3[:, c * 4:(c + 1) * 4, :], ysl,
                            )

            # ---- S2: K + softmax (exp/Z/normalize) interleaved with gate
            #      production for chunks 0..3
            sks = []
            for _g in range(2):
                sksb_g = skp.tile([128, NT * 512], BF16, tag="sksb")
                sks.append(sksb_g)
            gtiles = {}

            def gout_v_unit(c, u, gops, vps):
                """One head-pair unit of phase-B gate production for chunk c."""
                csl = slice(c * 512, (c + 1) * 512)
                g_ps = gops.tile([128, 1024], F32, tag="gout")
                v_ps = vps.tile([128, 1024], F32, tag="vp")
                for d in range(2):
                    h = u * 2 + d
                    nc.tensor.matmul(
                        g_ps[:, d * 512:(d + 1) * 512],
                        gqt[:, h * 128:(h + 1) * 128],
                        yq[:, csl], start=True, stop=True,
                        skip_group_check=True,
                    )
                for d in range(2):
                    h = u * 2 + d
                    nc.tensor.matmul(
                        v_ps[:, d * 512:(d + 1) * 512],
                        wtv[:, h * 128:(h + 1) * 128],
                        yv[:, csl], start=True, stop=True,
                        skip_group_check=True,
                    )
                th = sgp.tile([128, 1024], BF16, tag="sig")
                nc.scalar.activation(
                    th, g_ps, AF.Tanh, bias=biasp[:, 3:4], scale=0.5,
                )
                if c < 4:
                    gate = gp.tile([128, 1024], BF16, tag=f"gate{c}_{u}")
                else:
                    gate = gpj.tile([128, 1024], BF16, tag="gatejit")
                # gate = (tanh + 1) * (0.5*v)  == sigmoid(gout) * v
                nc.vector.scalar_tensor_tensor(
                    gate, th, 1.0, v_ps, ALU.add, ALU.mult,
                )
                gtiles[(c, u)] = gate

            def k_quad(g, q, kqps):
                osl = slice(g * 512, (g + 1) * 512)
                sksb = sks[g]
                t0 = q * 4
                kq = kqps.tile([128, 2048], F32, tag="kq")
                for i in range(4):
                    t = t0 + i
                    nc.tensor.matmul(
                        kq[:, i * 512:(i + 1) * 512],
                        yk[:, t * 128:(t + 1) * 128], wtk[:, osl],
                        start=True, stop=True, skip_group_check=True,
                    )
                nc.scalar.activation(
                    sksb[:, t0 * 512:(t0 + 4) * 512], kq, AF.Exp,
                )
                zb = g * NT * GH + t0 * GH
                with nc.allow_low_precision(reason="softmax Z in bf16"):
                    nc.vector.tensor_reduce(
                        zt[:, zb:zb + 4 * GH],
                        sksb[:, t0 * 512:(t0 + 4) * 512].rearrange(
                            "p (t h d) -> p t h d", t=4, h=GH
                        ),
                        mybir.AxisListType.X, ALU.add,
                    )
                nc.vector.reciprocal(zi[:, zb:zb + 4 * GH], zt[:, zb:zb + 4 * GH])
                # normalize the whole quad in one op: stride-0 broadcast of zi
                zsl = zi[:, zb:zb + 4 * GH]
                zbc = bass.AP(
                    zsl.tensor, zsl.offset, zsl.ap[:-1] + [[1, 4 * GH], [0, 128]]
                )
                qv = sksb[:, t0 * 512:(t0 + 4) * 512].rearrange(
                    "p (t j) -> p t j", t=4 * GH
                )
                nc.vector.tensor_tensor(qv, qv, zbc, ALU.mult)

            with (
                tc.tile_pool(name="kqps", bufs=1, space="PSUM") as kqps,
                tc.tile_pool(name="gops1", bufs=1, space="PSUM") as gops1,
                tc.tile_pool(name="vps1", bufs=1, space="PSUM") as vps1,
            ):
                # 16 K-quads round-robined with 16 gate units (chunks 0..3)
                for step in range(16):
                    g, q = divmod(step, NQ)
                    k_quad(g, q, kqps)
                    c, u = divmod(step, 4)
                    gout_v_unit(c, u, gops1, vps1)

            # ---- S3: R accumulation + kat chain -> W3^T
            with (
                tc.tile_pool(name="rps", bufs=2, space="PSUM") as rps,
                tc.tile_pool(name="katps", bufs=1, space="PSUM") as katps,
                tc.tile_pool(name="m2ps", bufs=1, space="PSUM") as m2ps,
                tc.tile_pool(name="w3ps", bufs=1, space="PSUM") as w3ps,
                tc.tile_pool(name="small", bufs=2) as sp,
            ):
                w3t_ps = w3ps.tile([128, 128], F32)
                for g in range(GROUPS):
                    sksb = sks[g]
                    r_ps = rps.tile([128, 512], F32, tag="r")
                    for t in range(NT):
                        nc.tensor.matmul(
                            r_ps, yqt[:, t * 128:(t + 1) * 128],
                            sksb[:, t * 512:(t + 1) * 512],
                            start=(t == 0), stop=(t == NT - 1),
                            skip_group_check=True,
                        )
                    r_sb = sp.tile([128, 512], BF16, tag="rsb")
                    nc.vector.tensor_copy(r_sb, r_ps)
                    for hh in range(GH):
                        h = g * GH + hh
                        kat_ps = katps.tile([128, 128], F32, tag="katp")
                        nc.tensor.matmul(
                            kat_ps, wtqr[:, h * 128:(h + 1) * 128],
                            r_sb[:, hh * 128:(hh + 1) * 128],
                            start=True, stop=True, skip_group_check=True,
                        )
                        kat_sb = sp.tile([128, 128], BF16, tag="katsb")
                        nc.vector.tensor_copy(kat_sb, kat_ps)
                        m2_ps = m2ps.tile([128, 128], F32, tag="m2")
                        nc.tensor.matmul(
                            m2_ps, kat_sb,
                            wvdm[:, h * 128:(h + 1) * 128],
                            start=True, stop=True, skip_group_check=True,
                        )
                        m2_sb = sp.tile([128, 128], BF16, tag="m2sb")
                        nc.vector.tensor_copy(m2_sb, m2_ps)
                        nc.tensor.matmul(
                            w3t_ps, m2_sb,
                            outwt[:, h * 128:(h + 1) * 128],
                            start=(h == 0), stop=(h == HEADS - 1),
                            skip_group_check=True,
                        )
                nc.vector.tensor_copy(w3t_sb, w3t_ps)

            # ---- S4: final projection; JIT gate production for chunks 4..7
            with (
                tc.tile_pool(name="gops2", bufs=2, space="PSUM") as gops2,
                tc.tile_pool(name="vps2", bufs=1, space="PSUM") as vps2,
                tc.tile_pool(name="finps", bufs=2, space="PSUM") as finps,
            ):
                def fin_chunk(c):
                    csl = slice(c * 512, (c + 1) * 512)
                    fin_ps = finps.tile([128, 512], F32, tag="fin")
                    for u in range(4):
                        gate = gtiles[(c, u)]
                        for d in range(2):
                            h = u * 2 + d
                            nc.tensor.matmul(
                                fin_ps, outwt[:, h * 128:(h + 1) * 128],
                                gate[:, d * 512:(d + 1) * 512],
                                start=(h == 0), stop=False,
                                skip_group_check=True,
                            )
                    nc.tensor.matmul(
                        fin_ps, w3t_sb, yv[:, csl],
                        start=False, stop=True, skip_group_check=True,
                    )
                    fin_sb = op_.tile([128, 512], F32, tag="finsb")
                    nc.scalar.activation(
                        fin_sb, fin_ps, AF.Identity, bias=biasp[:, 4:5],
                    )
                    nc.sync.dma_start(out=out_d[:, csl], in_=fin_sb)

                # produce chunks 4,5 gates first, then pipeline fins
                for u in range(4):
                    gout_v_unit(4, u, gops2, vps2)
                for u in range(4):
                    gout_v_unit(5, u, gops2, vps2)
                fin_chunk(4)
                for u in range(4):
                    gout_v_unit(6, u, gops2, vps2)
                fin_chunk(5)
                for u in range(4):
                    gout_v_unit(7, u, gops2, vps2)
                for c in (6, 7, 0, 1, 2, 3):
                    fin_chunk(c)

    nc.compile()
    return nc


def kernel(**inputs):
    global _NC, LAST_EXEC_NS
    host = _prep(inputs)
    if _NC is None:
        _NC = _build()
    x = np.asarray(inputs["x"], np.float32)
    in_maps = []
    for b in range(B):
        xp = np.pad(x[b], ((0, 0), (1, 1)))
        m = {"xb": _bf(xp)}
        m.update(host)
        in_maps.append(m)
    res = run_bass_kernel_spmd(
        _NC, in_maps, core_ids=list(range(B)), trace=TRACE
    )
    LAST_EXEC_NS = res.exec_time_ns
    return np.stack([r["out"] for r in res.results]).astype(np.float32)
